# revision 13
# baseline (speedup 1.0000x reference)
"""YOLO-style DetectionLoss on 8 Trainium2 NeuronCores (Bass/Tile), v3.

Pure data parallelism: batch 8192 -> 1024 per core; 1024*49 = 50176 cells
as 128 partitions x 392 cells. All tiles use the natural [P, k, box, f]
layout (no transposed engine reads - measured ~2x penalty on ACT for
strided reads). Masks read via zero-stride broadcasts (measured free on
DVE). Intermediates in bf16 (DVE 2x/4x modes confirmed on HW), the
IoU-division tail in f32 (reciprocal_approx_fast is f32-only).

Engine split (measured HW rates: DVE ~.56us/elem-unit 1x, .28 2x;
ACT ~.46 contiguous; Pool ~.7-1.0 + high per-instr overhead):
  Pool: dxy diff, class sub, a share of the class conf-mask.
  ACT : 2*twh0 / 4*twh0 broadcast materializations, sqrt(pwh), sqrt(twh),
        and ONE fused Square+accumulate per chunk over the combined
        pre-weighted tile SQT = [cwx | cwh | dcm | pcm | mdcl].
  DVE : the overlap chain, IoU, responsible-box mask, mask-muls, the
        rest of the class mask.

Term weights (5, 1, 0.5, 1) are folded into the masks (sqrt5, sqrt.5),
so one f32 accumulator column per chunk holds the full weighted sum;
the host sums and divides by B.
"""

import numpy as np

import concourse.bacc as bacc
import concourse.mybir as mybir
import concourse.tile as tile
from concourse.bass_utils import run_bass_kernel_spmd

F32 = mybir.dt.float32
BF16 = mybir.dt.bfloat16
AF = mybir.ActivationFunctionType
OP = mybir.AluOpType
AX = mybir.AxisListType

NB, C, S = 3, 20, 7
D = 5 * NB + C                 # 35
B = 8192
NCORES = 8
P = 128

SQ5 = 5.0 ** 0.5
SQH = 0.5 ** 0.5
BIG = 1000.0

# class conf-mask channel split: [0:CLS_POOL) on Pool, rest on DVE
CLS_POOL = 12


def default_chunks(kpp):
    if kpp % 98 == 0:
        return [98] * (kpp // 98)
    return [kpp]


def build_nc(bc: int, ks=None, io_bufs: int = 3, loop_repeats: int = 0,
             cls_pool: int = CLS_POOL, repeats: int = 1, wk_bufs: int = 3,
             debug_sqt: bool = False):
    cells = bc * S * S
    assert cells % P == 0
    kpp = cells // P
    if ks is None:
        ks = default_chunks(kpp)
    assert sum(ks) == kpp
    nchunks = len(ks)

    nc = bacc.Bacc("TRN2", debug=False, num_devices=NCORES)
    out_h = nc.dram_tensor("output", [bc, S, S, D], F32, kind="ExternalInput")
    tgt_h = nc.dram_tensor("target", [bc, S, S, D], F32, kind="ExternalInput")
    acc_h = nc.dram_tensor("acc", [P, nchunks], F32, kind="ExternalOutput")
    sqt_h = (nc.dram_tensor("sqtd", [P, ks[0] * 38], F32, kind="ExternalOutput")
             if debug_sqt else None)

    out_v = out_h.ap().rearrange("(p a) h w d -> p (a h w d)", p=P)
    tgt_v = tgt_h.ap().rearrange("(p a) h w d -> p (a h w d)", p=P)

    with tile.TileContext(nc) as tc:
        with (
            tc.tile_pool(name="io", bufs=io_bufs) as io_pool,
            tc.tile_pool(name="wk", bufs=wk_bufs) as wk,
            tc.tile_pool(name="accp", bufs=1) as accp,
        ):
            acc = accp.tile([P, nchunks], F32, name="acc")

            import contextlib
            loop_cm = (tc.For_i(0, loop_repeats, 1) if loop_repeats
                       else contextlib.nullcontext())
            with loop_cm:
              for _rep in range(repeats):
                off = 0
                for ci, k in enumerate(ks):
                    ot = io_pool.tile([P, k * D], F32, name="ot", tag="ot")
                    tt = io_pool.tile([P, k * D], F32, name="tt", tag="tt")
                    nc.sync.dma_start(ot[:], out_v[:, off:off + k * D])
                    nc.sync.dma_start(tt[:], tgt_v[:, off:off + k * D])
                    off += k * D

                    o3 = ot[:].rearrange("p (k d) -> p k d", d=D)
                    t3 = tt[:].rearrange("p (k d) -> p k d", d=D)
                    ob = o3[:, :, 0:15].rearrange("p k (b f) -> p k b f", f=5)
                    tb = t3[:, :, 0:15].rearrange("p k (b f) -> p k b f", f=5)

                    pxy = ob[:, :, :, 0:2]          # [P,k,3,2]
                    pwh = ob[:, :, :, 2:4]
                    twh = tb[:, :, :, 2:4]
                    pcls = o3[:, :, 15:35]
                    tcls = t3[:, :, 15:35]
                    txy0 = t3[:, :, 0:2]            # [P,k,2]
                    twh0 = t3[:, :, 2:4]
                    confv = t3[:, :, 4]             # [P,k] 0/1

                    def WT(shape, dt, name):
                        return wk.tile(shape, dt, name=name, tag=name)[:]

                    # --- tiles (natural [P,k,3,2] box layout) ---
                    t0w2 = WT([P, k, 3, 2], BF16, "t0w2")   # 2*twh0 bcast
                    t0w4 = WT([P, k, 3, 2], BF16, "t0w4")   # 4*twh0 bcast
                    sp = WT([P, k, 3, 2], BF16, "sp")
                    st = WT([P, k, 3, 2], BF16, "st")
                    dwh = WT([P, k, 3, 2], BF16, "dwh")
                    dxy = WT([P, k, 3, 2], BF16, "dxy")
                    dcx = WT([P, k, 3, 2], BF16, "dcx")
                    acx = WT([P, k, 3, 2], BF16, "acx")
                    s6d = WT([P, k, 3, 2], BF16, "s6d")
                    mn4 = WT([P, k, 3, 2], BF16, "mn4")
                    u6 = WT([P, k, 3, 2], BF16, "u6")
                    ovf = WT([P, k, 3, 2], BF16, "ovf")
                    ovr = WT([P, k, 3, 2], BF16, "ovr")
                    dclb = WT([P, k, 20], BF16, "dclb")
                    sqt = WT([P, k, 38], BF16, "sqt")
                    inter = WT([P, k, 3], BF16, "inter")
                    rmb = WT([P, k, 3], BF16, "rmb")
                    rm5 = WT([P, k, 3], BF16, "rm5")
                    a1 = WT([P, k, 3], F32, "a1")
                    a12 = WT([P, k, 3], F32, "a12")
                    den = WT([P, k, 3], F32, "den")
                    rcp = WT([P, k, 3], F32, "rcp")
                    iou = WT([P, k, 3], F32, "iou")
                    dct = WT([P, k, 3], F32, "dct")
                    a2 = WT([P, k], F32, "a2")
                    cc = WT([P, k], F32, "cc")
                    nb = WT([P, k], F32, "nb")
                    mxc = WT([P, k], F32, "mxc")
                    mx = WT([P, k], F32, "mx")
                    notc = WT([P, k], F32, "notc")

                    cwx = sqt[:, :, 0:6].rearrange("p k (b f) -> p k b f", f=2)
                    cwh = sqt[:, :, 6:12].rearrange("p k (b f) -> p k b f", f=2)
                    sdcm = sqt[:, :, 12:15]
                    smdcl = sqt[:, :, 18:38]

                    # ---------- ACT: contiguous-ish unary mats ----------
                    t0wb = twh0.unsqueeze(2).broadcast_to([P, k, 3, 2])
                    nc.scalar.activation(t0w2, t0wb, AF.Copy, scale=2.0)
                    nc.scalar.activation(t0w4, t0wb, AF.Copy, scale=4.0)
                    nc.scalar.activation(sp, pwh, AF.Sqrt)
                    nc.scalar.activation(st, twh, AF.Sqrt)

                    # ---------- Pool: first-touch ----------
                    nc.gpsimd.tensor_sub(dxy, pxy, tb[:, :, :, 0:2])
                    nc.gpsimd.tensor_sub(dclb, pcls, tcls)

                    # ---------- DVE ----------
                    nc.vector.tensor_copy(cc, confv)
                    nc.vector.tensor_sub(
                        dcx, pxy, txy0.unsqueeze(2).broadcast_to([P, k, 3, 2]))
                    # acx = |dcx| * 4/S   (contiguous ACT read of own tile)
                    nc.scalar.activation(acx, dcx, AF.Abs, scale=4.0 / S)
                    # s6d = 2pw + 2tw ; mn4 = min(4pw, 4tw)  (per-f 3D STTs)
                    for f in range(2):
                        nc.vector.scalar_tensor_tensor(
                            s6d[:, :, :, f], pwh[:, :, :, f], 2.0,
                            t0w2[:, :, :, f], op0=OP.mult, op1=OP.add)
                        nc.vector.scalar_tensor_tensor(
                            mn4[:, :, :, f], pwh[:, :, :, f], 4.0,
                            t0w4[:, :, :, f], op0=OP.mult, op1=OP.min)
                    # u6 = s6d - acx = 4*(half-sum - |dc|) ; ov = min(mn4,u6)
                    nc.vector.tensor_sub(u6, s6d, acx)
                    nc.vector.tensor_tensor(ovf, mn4, u6, op=OP.min)
                    nc.vector.tensor_scalar_max(ovr, ovf, 0.0)
                    nc.vector.tensor_mul(inter, ovr[:, :, :, 0],
                                         ovr[:, :, :, 1])

                    # areas (x16 to match inter = 16*true), den, iou
                    nc.vector.scalar_tensor_tensor(
                        a1, ob[:, :, :, 2], 16.0, ob[:, :, :, 3],
                        op0=OP.mult, op1=OP.mult)
                    nc.vector.scalar_tensor_tensor(
                        a2, twh0[:, :, 0], 16.0, twh0[:, :, 1],
                        op0=OP.mult, op1=OP.mult)
                    nc.vector.tensor_add(
                        a12, a1, a2.unsqueeze(2).broadcast_to([P, k, 3]))
                    nc.vector.tensor_sub(den, a12, inter)
                    nc.vector.reciprocal_approx_fast(
                        rcp.rearrange("p k b -> p (k b)"),
                        den.rearrange("p k b -> p (k b)"))
                    nc.vector.tensor_mul(iou, inter, rcp)

                    # responsible-box mask: rm = (iou >= mx + (cc!=1)*BIG)
                    nc.vector.tensor_reduce(mx, iou, axis=AX.X, op=OP.max)
                    nc.vector.tensor_scalar(
                        nb, cc, 1.0, BIG, op0=OP.not_equal, op1=OP.mult)
                    nc.vector.tensor_add(mxc, mx, nb)
                    nc.vector.tensor_tensor(
                        rmb, iou, mxc.unsqueeze(2).broadcast_to([P, k, 3]),
                        op=OP.is_ge)
                    nc.vector.tensor_scalar_mul(rm5, rmb, SQ5)
                    rm5b = rm5.unsqueeze(3).broadcast_to([P, k, 3, 2])

                    # ---------- masked residuals into SQT ----------
                    nc.vector.tensor_mul(cwx, dxy, rm5b)
                    nc.vector.tensor_sub(dwh, sp, st)
                    nc.vector.tensor_mul(cwh, dwh, rm5b)
                    nc.vector.tensor_sub(
                        dct, ob[:, :, :, 4],
                        mx.unsqueeze(2).broadcast_to([P, k, 3]))
                    nc.vector.tensor_mul(sdcm, dct, rmb)
                    nc.vector.tensor_scalar(
                        notc, cc, 1.0, SQH, op0=OP.not_equal, op1=OP.mult)
                    nc.vector.tensor_mul(
                        sqt[:, :, 15:18], ob[:, :, :, 4],
                        notc.unsqueeze(2).broadcast_to([P, k, 3]))
                    # class conf-mask, split channel-wise Pool/DVE
                    cp = cls_pool
                    ccb = cc.unsqueeze(2)
                    if cp > 0:
                        nc.gpsimd.tensor_mul(
                            smdcl[:, :, 0:cp], dclb[:, :, 0:cp],
                            ccb.broadcast_to([P, k, cp]))
                    if cp < 20:
                        nc.vector.tensor_mul(
                            smdcl[:, :, cp:20], dclb[:, :, cp:20],
                            ccb.broadcast_to([P, k, 20 - cp]))

                    if debug_sqt and ci == 0:
                        sq32 = WT([P, k, 38], F32, "sq32")
                        nc.vector.tensor_copy(sq32, sqt)
                        nc.sync.dma_start(
                            sqt_h.ap()[:],
                            sq32.rearrange("p k d -> p (k d)"))
                    # ---------- ACT: single fused Square+accumulate ----
                    nc.scalar.activation(sqt, sqt, AF.Square,
                                         accum_out=acc[:, ci:ci + 1])

            nc.sync.dma_start(acc_h.ap()[:], acc[:])

    nc.compile()
    return nc


_CACHE = {}


def _get_nc(bc, ks=None, io_bufs=3, loop_repeats=0, cls_pool=CLS_POOL,
            repeats=1, **_ignored):
    key = (bc, tuple(ks) if ks else None, io_bufs, loop_repeats, cls_pool,
           repeats)
    if key not in _CACHE:
        _CACHE[key] = build_nc(bc, ks, io_bufs, loop_repeats, cls_pool,
                               repeats)
    return _CACHE[key]


def combine_acc(acc_list, nchunks):
    tot = np.float64(0.0)
    for a in acc_list:
        tot += a.astype(np.float64).sum()
    return np.float32(tot / B)


BEST_KS = [98, 98, 98, 98]
BEST_IO_BUFS = 3


def extra_inputs():
    return {}


def kernel(output: np.ndarray, target: np.ndarray) -> np.ndarray:
    assert output.shape == (B, S, S, D) and target.shape == (B, S, S, D)
    bc = B // NCORES
    nchunks = len(BEST_KS)
    nc = _get_nc(bc, BEST_KS, io_bufs=BEST_IO_BUFS)
    in_maps = [
        {
            "output": np.ascontiguousarray(output[i * bc:(i + 1) * bc]),
            "target": np.ascontiguousarray(target[i * bc:(i + 1) * bc]),
        }
        for i in range(NCORES)
    ]
    res = run_bass_kernel_spmd(nc, in_maps, list(range(NCORES)))
    return combine_acc([r["acc"] for r in res.results], nchunks)


# revision 18
# speedup vs baseline: 1.0948x; 1.0948x over previous
"""YOLO-style DetectionLoss on 8 Trainium2 NeuronCores (Bass/Tile), v3.

Pure data parallelism: batch 8192 -> 1024 per core; 1024*49 = 50176 cells
as 128 partitions x 392 cells. All tiles use the natural [P, k, box, f]
layout (no transposed engine reads - measured ~2x penalty on ACT for
strided reads). Masks read via zero-stride broadcasts (measured free on
DVE). Intermediates in bf16 (DVE 2x/4x modes confirmed on HW), the
IoU-division tail in f32 (reciprocal_approx_fast is f32-only).

Engine split (measured HW rates: DVE ~.56us/elem-unit 1x, .28 2x;
ACT ~.46 contiguous; Pool ~.7-1.0 + high per-instr overhead):
  Pool: dxy diff, class sub, a share of the class conf-mask.
  ACT : 2*twh0 / 4*twh0 broadcast materializations, sqrt(pwh), sqrt(twh),
        and ONE fused Square+accumulate per chunk over the combined
        pre-weighted tile SQT = [cwx | cwh | dcm | pcm | mdcl].
  DVE : the overlap chain, IoU, responsible-box mask, mask-muls, the
        rest of the class mask.

Term weights (5, 1, 0.5, 1) are folded into the masks (sqrt5, sqrt.5),
so one f32 accumulator column per chunk holds the full weighted sum;
the host sums and divides by B.
"""

import numpy as np

import concourse.bacc as bacc
import concourse.mybir as mybir
import concourse.tile as tile
from concourse.bass_utils import run_bass_kernel_spmd

F32 = mybir.dt.float32
BF16 = mybir.dt.bfloat16
AF = mybir.ActivationFunctionType
OP = mybir.AluOpType
AX = mybir.AxisListType

NB, C, S = 3, 20, 7
D = 5 * NB + C                 # 35
B = 8192
NCORES = 8
P = 128

SQ5 = 5.0 ** 0.5
SQH = 0.5 ** 0.5
BIG = 1000.0

# class conf-mask channel split: [0:CLS_POOL) on Pool, rest on DVE
CLS_POOL = 12


def default_chunks(kpp):
    if kpp % 98 == 0:
        return [98] * (kpp // 98)
    return [kpp]


def build_nc(bc: int, ks=None, io_bufs: int = 3, loop_repeats: int = 0,
             cls_pool: int = CLS_POOL, repeats: int = 1, wk_bufs: int = 3,
             debug_sqt: bool = False):
    cells = bc * S * S
    assert cells % P == 0
    kpp = cells // P
    if ks is None:
        ks = default_chunks(kpp)
    assert sum(ks) == kpp
    nchunks = len(ks)

    nc = bacc.Bacc("TRN2", debug=False, num_devices=NCORES)
    out_h = nc.dram_tensor("output", [bc, S, S, D], F32, kind="ExternalInput")
    tgt_h = nc.dram_tensor("target", [bc, S, S, D], F32, kind="ExternalInput")
    acc_h = nc.dram_tensor("acc", [P, nchunks], F32, kind="ExternalOutput")
    sqt_h = (nc.dram_tensor("sqtd", [P, ks[0] * 38], F32, kind="ExternalOutput")
             if debug_sqt else None)

    out_v = out_h.ap().rearrange("(p a) h w d -> p (a h w d)", p=P)
    tgt_v = tgt_h.ap().rearrange("(p a) h w d -> p (a h w d)", p=P)

    with tile.TileContext(nc) as tc:
        with (
            tc.tile_pool(name="io", bufs=io_bufs) as io_pool,
            tc.tile_pool(name="wk", bufs=wk_bufs) as wk,
            tc.tile_pool(name="accp", bufs=1) as accp,
        ):
            acc = accp.tile([P, nchunks], F32, name="acc")

            import contextlib
            # Software-pipeline across iterations: unroll 2 passes per
            # For_i iteration so the static schedule overlaps the drain of
            # pass i with the ramp of pass i+1 (total passes unchanged).
            if loop_repeats and loop_repeats % 2 == 0 and repeats == 1:
                loop_repeats //= 2
                repeats = 2
            loop_cm = (tc.For_i(0, loop_repeats, 1) if loop_repeats
                       else contextlib.nullcontext())
            with loop_cm:
              for _rep in range(repeats):
                off = 0
                for ci, k in enumerate(ks):
                    ot = io_pool.tile([P, k * D], F32, name="ot", tag="ot")
                    tt = io_pool.tile([P, k * D], F32, name="tt", tag="tt")
                    nc.sync.dma_start(ot[:], out_v[:, off:off + k * D])
                    nc.sync.dma_start(tt[:], tgt_v[:, off:off + k * D])
                    off += k * D

                    o3 = ot[:].rearrange("p (k d) -> p k d", d=D)
                    t3 = tt[:].rearrange("p (k d) -> p k d", d=D)
                    ob = o3[:, :, 0:15].rearrange("p k (b f) -> p k b f", f=5)
                    tb = t3[:, :, 0:15].rearrange("p k (b f) -> p k b f", f=5)

                    pxy = ob[:, :, :, 0:2]          # [P,k,3,2]
                    pwh = ob[:, :, :, 2:4]
                    twh = tb[:, :, :, 2:4]
                    pcls = o3[:, :, 15:35]
                    tcls = t3[:, :, 15:35]
                    txy0 = t3[:, :, 0:2]            # [P,k,2]
                    twh0 = t3[:, :, 2:4]
                    confv = t3[:, :, 4]             # [P,k] 0/1

                    def WT(shape, dt, name):
                        return wk.tile(shape, dt, name=name, tag=name)[:]

                    # --- tiles (natural [P,k,3,2] box layout) ---
                    t0w2 = WT([P, k, 3, 2], BF16, "t0w2")   # 2*twh0 bcast
                    t0w4 = WT([P, k, 3, 2], BF16, "t0w4")   # 4*twh0 bcast
                    sp = WT([P, k, 3, 2], BF16, "sp")
                    st = WT([P, k, 3, 2], BF16, "st")
                    dwh = WT([P, k, 3, 2], BF16, "dwh")
                    dxy = WT([P, k, 3, 2], BF16, "dxy")
                    dcx = WT([P, k, 3, 2], BF16, "dcx")
                    acx = WT([P, k, 3, 2], BF16, "acx")
                    s6d = WT([P, k, 3, 2], BF16, "s6d")
                    mn4 = WT([P, k, 3, 2], BF16, "mn4")
                    u6 = WT([P, k, 3, 2], BF16, "u6")
                    ovf = WT([P, k, 3, 2], BF16, "ovf")
                    ovr = WT([P, k, 3, 2], BF16, "ovr")
                    dclb = WT([P, k, 20], BF16, "dclb")
                    sqt = WT([P, k, 38], BF16, "sqt")
                    inter = WT([P, k, 3], BF16, "inter")
                    rmb = WT([P, k, 3], BF16, "rmb")
                    rm6 = WT([P, k, 3, 2], BF16, "rm6")
                    a1 = WT([P, k, 3], F32, "a1")
                    a12 = WT([P, k, 3], F32, "a12")
                    den = WT([P, k, 3], F32, "den")
                    rcp = WT([P, k, 3], F32, "rcp")
                    iou = WT([P, k, 3], F32, "iou")
                    dct = WT([P, k, 3], F32, "dct")
                    a2 = WT([P, k], F32, "a2")
                    cc = WT([P, k], F32, "cc")
                    nb = WT([P, k], F32, "nb")
                    mxc = WT([P, k], F32, "mxc")
                    mx = WT([P, k], F32, "mx")
                    notc = WT([P, k], F32, "notc")

                    cwx = sqt[:, :, 0:6].rearrange("p k (b f) -> p k b f", f=2)
                    cwh = sqt[:, :, 6:12].rearrange("p k (b f) -> p k b f", f=2)
                    sdcm = sqt[:, :, 12:15]
                    smdcl = sqt[:, :, 18:38]

                    # ---------- ACT: contiguous-ish unary mats ----------
                    t0wb = twh0.unsqueeze(2).broadcast_to([P, k, 3, 2])
                    nc.scalar.activation(t0w2, t0wb, AF.Copy, scale=2.0)
                    nc.vector.tensor_scalar_mul(t0w4, t0w2, 2.0)
                    nc.scalar.activation(sp, pwh, AF.Sqrt)
                    nc.scalar.activation(st, twh, AF.Sqrt)

                    # ---------- Pool: first-touch ----------
                    nc.gpsimd.tensor_sub(dxy, pxy, tb[:, :, :, 0:2])
                    nc.gpsimd.tensor_sub(dclb, pcls, tcls)

                    # ---------- DVE ----------
                    nc.vector.tensor_copy(cc, confv)
                    nc.vector.tensor_sub(
                        dcx, pxy, txy0.unsqueeze(2).broadcast_to([P, k, 3, 2]))
                    # acx = |dcx| * 4/S   (contiguous ACT read of own tile)
                    nc.scalar.activation(acx, dcx, AF.Abs, scale=4.0 / S)
                    # s6d = 2pw + 2tw ; mn4 = min(4pw, 4tw)  (per-f 3D STTs)
                    for f in range(2):
                        nc.vector.scalar_tensor_tensor(
                            s6d[:, :, :, f], pwh[:, :, :, f], 2.0,
                            t0w2[:, :, :, f], op0=OP.mult, op1=OP.add)
                        nc.vector.scalar_tensor_tensor(
                            mn4[:, :, :, f], pwh[:, :, :, f], 4.0,
                            t0w4[:, :, :, f], op0=OP.mult, op1=OP.min)
                    # u6 = s6d - acx = 4*(half-sum - |dc|) ; ov = min(mn4,u6)
                    nc.vector.tensor_sub(u6, s6d, acx)
                    nc.vector.tensor_tensor(ovf, mn4, u6, op=OP.min)
                    nc.vector.tensor_scalar_max(ovr, ovf, 0.0)
                    nc.vector.tensor_mul(inter, ovr[:, :, :, 0],
                                         ovr[:, :, :, 1])

                    # areas (x16 to match inter = 16*true), den, iou
                    nc.vector.scalar_tensor_tensor(
                        a1, ob[:, :, :, 2], 16.0, ob[:, :, :, 3],
                        op0=OP.mult, op1=OP.mult)
                    nc.vector.scalar_tensor_tensor(
                        a2, twh0[:, :, 0], 16.0, twh0[:, :, 1],
                        op0=OP.mult, op1=OP.mult)
                    nc.vector.tensor_add(
                        a12, a1, a2.unsqueeze(2).broadcast_to([P, k, 3]))
                    nc.vector.tensor_sub(den, a12, inter)
                    nc.vector.reciprocal_approx_fast(
                        rcp.rearrange("p k b -> p (k b)"),
                        den.rearrange("p k b -> p (k b)"))
                    nc.vector.tensor_mul(iou, inter, rcp)

                    # responsible-box mask: rm = (iou >= mx + (cc!=1)*BIG)
                    nc.vector.tensor_reduce(mx, iou, axis=AX.X, op=OP.max)
                    nc.vector.tensor_scalar(
                        nb, cc, 1.0, BIG, op0=OP.not_equal, op1=OP.mult)
                    nc.vector.tensor_add(mxc, mx, nb)
                    nc.vector.tensor_tensor(
                        rmb, iou, mxc.unsqueeze(2).broadcast_to([P, k, 3]),
                        op=OP.is_ge)
                    # packed sqrt(5)-scaled mask (2x-mode mat, enables 2x
                    # TT for the cw mask-muls)
                    nc.vector.tensor_scalar_mul(
                        rm6, rmb.unsqueeze(3).broadcast_to([P, k, 3, 2]), SQ5)

                    # ---------- masked residuals into SQT ----------
                    nc.vector.tensor_mul(cwx, dxy, rm6)
                    nc.vector.tensor_sub(dwh, sp, st)
                    nc.vector.tensor_mul(cwh, dwh, rm6)
                    nc.vector.tensor_sub(
                        dct, ob[:, :, :, 4],
                        mx.unsqueeze(2).broadcast_to([P, k, 3]))
                    nc.vector.tensor_mul(sdcm, dct, rmb)
                    nc.vector.tensor_scalar(
                        notc, cc, 1.0, SQH, op0=OP.not_equal, op1=OP.mult)
                    nc.vector.tensor_mul(
                        sqt[:, :, 15:18], ob[:, :, :, 4],
                        notc.unsqueeze(2).broadcast_to([P, k, 3]))
                    # class conf-mask, split channel-wise Pool/DVE
                    cp = cls_pool
                    ccb = cc.unsqueeze(2)
                    if cp > 0:
                        nc.gpsimd.tensor_mul(
                            smdcl[:, :, 0:cp], dclb[:, :, 0:cp],
                            ccb.broadcast_to([P, k, cp]))
                    if cp < 20:
                        nc.vector.tensor_mul(
                            smdcl[:, :, cp:20], dclb[:, :, cp:20],
                            ccb.broadcast_to([P, k, 20 - cp]))

                    if debug_sqt and ci == 0:
                        sq32 = WT([P, k, 38], F32, "sq32")
                        nc.vector.tensor_copy(sq32, sqt)
                        nc.sync.dma_start(
                            sqt_h.ap()[:],
                            sq32.rearrange("p k d -> p (k d)"))
                    # ---------- ACT: single fused Square+accumulate ----
                    nc.scalar.activation(sqt, sqt, AF.Square,
                                         accum_out=acc[:, ci:ci + 1])

            nc.sync.dma_start(acc_h.ap()[:], acc[:])

    nc.compile()
    return nc


_CACHE = {}


def _get_nc(bc, ks=None, io_bufs=3, loop_repeats=0, cls_pool=CLS_POOL,
            repeats=1, **_ignored):
    key = (bc, tuple(ks) if ks else None, io_bufs, loop_repeats, cls_pool,
           repeats)
    if key not in _CACHE:
        _CACHE[key] = build_nc(bc, ks, io_bufs, loop_repeats, cls_pool,
                               repeats)
    return _CACHE[key]


def combine_acc(acc_list, nchunks):
    tot = np.float64(0.0)
    for a in acc_list:
        tot += a.astype(np.float64).sum()
    return np.float32(tot / B)


BEST_KS = [49, 98, 98, 98, 49]
BEST_IO_BUFS = 3


def extra_inputs():
    return {}


def kernel(output: np.ndarray, target: np.ndarray) -> np.ndarray:
    assert output.shape == (B, S, S, D) and target.shape == (B, S, S, D)
    bc = B // NCORES
    nchunks = len(BEST_KS)
    nc = _get_nc(bc, BEST_KS, io_bufs=BEST_IO_BUFS)
    in_maps = [
        {
            "output": np.ascontiguousarray(output[i * bc:(i + 1) * bc]),
            "target": np.ascontiguousarray(target[i * bc:(i + 1) * bc]),
        }
        for i in range(NCORES)
    ]
    res = run_bass_kernel_spmd(nc, in_maps, list(range(NCORES)))
    return combine_acc([r["acc"] for r in res.results], nchunks)


# revision 27
# speedup vs baseline: 1.1926x; 1.0893x over previous
"""YOLO-style DetectionLoss on 8 Trainium2 NeuronCores (Bass/Tile), v3.

Pure data parallelism: batch 8192 -> 1024 per core; 1024*49 = 50176 cells
as 128 partitions x 392 cells. All tiles use the natural [P, k, box, f]
layout (no transposed engine reads - measured ~2x penalty on ACT for
strided reads). Masks read via zero-stride broadcasts (measured free on
DVE). Intermediates in bf16 (DVE 2x/4x modes confirmed on HW), the
IoU-division tail in f32 (reciprocal_approx_fast is f32-only).

Engine split (measured HW rates: DVE ~.56us/elem-unit 1x, .28 2x;
ACT ~.46 contiguous; Pool ~.7-1.0 + high per-instr overhead):
  Pool: dxy diff, class sub, a share of the class conf-mask.
  ACT : 2*twh0 / 4*twh0 broadcast materializations, sqrt(pwh), sqrt(twh),
        and ONE fused Square+accumulate per chunk over the combined
        pre-weighted tile SQT = [cwx | cwh | dcm | pcm | mdcl].
  DVE : the overlap chain, IoU, responsible-box mask, mask-muls, the
        rest of the class mask.

Term weights (5, 1, 0.5, 1) are folded into the masks (sqrt5, sqrt.5),
so one f32 accumulator column per chunk holds the full weighted sum;
the host sums and divides by B.
"""

import numpy as np

import concourse.bacc as bacc
import concourse.mybir as mybir
import concourse.tile as tile
from concourse.bass_utils import run_bass_kernel_spmd

F32 = mybir.dt.float32
BF16 = mybir.dt.bfloat16
AF = mybir.ActivationFunctionType
OP = mybir.AluOpType
AX = mybir.AxisListType

NB, C, S = 3, 20, 7
D = 5 * NB + C                 # 35
B = 8192
NCORES = 8
P = 128

SQ5 = 5.0 ** 0.5
SQH = 0.5 ** 0.5
BIG = 1000.0

# class conf-mask channel split: [0:CLS_POOL) on Pool, rest on DVE
CLS_POOL = 12


def default_chunks(kpp):
    if kpp % 98 == 0:
        return [98] * (kpp // 98)
    return [kpp]


def build_nc(bc: int, ks=None, io_bufs: int = 3, loop_repeats: int = 0,
             cls_pool: int = CLS_POOL, repeats: int = 1, wk_bufs: int = 3,
             debug_sqt: bool = False, ablate: str | None = None):
    cells = bc * S * S
    assert cells % P == 0
    kpp = cells // P
    if ks is None:
        ks = default_chunks(kpp)
    assert sum(ks) == kpp
    nchunks = len(ks)

    nc = bacc.Bacc("TRN2", debug=False, num_devices=NCORES)
    out_h = nc.dram_tensor("output", [bc, S, S, D], F32, kind="ExternalInput")
    tgt_h = nc.dram_tensor("target", [bc, S, S, D], F32, kind="ExternalInput")
    acc_h = nc.dram_tensor("acc", [P, nchunks], F32, kind="ExternalOutput")
    sqt_h = (nc.dram_tensor("sqtd", [P, ks[0] * 38], F32, kind="ExternalOutput")
             if debug_sqt else None)

    out_v = out_h.ap().rearrange("(p a) h w d -> p (a h w d)", p=P)
    tgt_v = tgt_h.ap().rearrange("(p a) h w d -> p (a h w d)", p=P)

    with tile.TileContext(nc) as tc:
        with (
            tc.tile_pool(name="io", bufs=io_bufs) as io_pool,
            tc.tile_pool(name="wk", bufs=wk_bufs) as wk,
            tc.tile_pool(name="accp", bufs=1) as accp,
        ):
            acc = accp.tile([P, nchunks], F32, name="acc")

            import contextlib
            # Software-pipeline across iterations: unroll 2 passes per
            # For_i iteration so the static schedule overlaps the drain of
            # pass i with the ramp of pass i+1 (total passes unchanged).
            if loop_repeats and loop_repeats % 2 == 0 and repeats == 1:
                loop_repeats //= 2
                repeats = 2
            loop_cm = (tc.For_i(0, loop_repeats, 1) if loop_repeats
                       else contextlib.nullcontext())
            with loop_cm:
              for _rep in range(repeats):
                off = 0
                for ci, k in enumerate(ks):
                    ot = io_pool.tile([P, k * D], F32, name="ot", tag="ot")
                    tt = io_pool.tile([P, k * D], F32, name="tt", tag="tt")
                    nc.sync.dma_start(ot[:], out_v[:, off:off + k * D])
                    nc.sync.dma_start(tt[:], tgt_v[:, off:off + k * D])
                    off += k * D

                    o3 = ot[:].rearrange("p (k d) -> p k d", d=D)
                    t3 = tt[:].rearrange("p (k d) -> p k d", d=D)
                    ob = o3[:, :, 0:15].rearrange("p k (b f) -> p k b f", f=5)
                    tb = t3[:, :, 0:15].rearrange("p k (b f) -> p k b f", f=5)

                    pxy = ob[:, :, :, 0:2]          # [P,k,3,2]
                    pwh = ob[:, :, :, 2:4]
                    twh = tb[:, :, :, 2:4]
                    pcls = o3[:, :, 15:35]
                    tcls = t3[:, :, 15:35]
                    txy0 = t3[:, :, 0:2]            # [P,k,2]
                    twh0 = t3[:, :, 2:4]
                    confv = t3[:, :, 4]             # [P,k] 0/1

                    def WT(shape, dt, name):
                        return wk.tile(shape, dt, name=name, tag=name)[:]

                    # --- tiles (natural [P,k,3,2] box layout) ---
                    t0w2 = WT([P, k, 3, 2], BF16, "t0w2")   # 2*twh0 bcast
                    t0w4 = WT([P, k, 3, 2], BF16, "t0w4")   # 4*twh0 bcast
                    p2w = WT([P, k, 3, 2], BF16, "p2w")     # 2*pwh
                    p4w = WT([P, k, 3, 2], BF16, "p4w")     # 4*pwh
                    sp = WT([P, k, 3, 2], BF16, "sp")
                    st = WT([P, k, 3, 2], BF16, "st")
                    dwh = WT([P, k, 3, 2], BF16, "dwh")
                    dxy = WT([P, k, 3, 2], BF16, "dxy")
                    dcx = WT([P, k, 3, 2], BF16, "dcx")
                    acx = WT([P, k, 3, 2], BF16, "acx")
                    s6d = WT([P, k, 3, 2], BF16, "s6d")
                    mn4 = WT([P, k, 3, 2], BF16, "mn4")
                    u6 = WT([P, k, 3, 2], BF16, "u6")
                    ovf = WT([P, k, 3, 2], BF16, "ovf")
                    ovr = WT([P, k, 3, 2], BF16, "ovr")
                    dclb = WT([P, k, 20], BF16, "dclb")
                    sqt = WT([P, k, 38], BF16, "sqt")
                    inter = WT([P, k, 3], BF16, "inter")
                    rmb = WT([P, k, 3], BF16, "rmb")
                    rm6 = WT([P, k, 3, 2], BF16, "rm6")
                    a1 = WT([P, k, 3], F32, "a1")
                    a12 = WT([P, k, 3], F32, "a12")
                    den = WT([P, k, 3], F32, "den")
                    rcp = WT([P, k, 3], F32, "rcp")
                    iou = WT([P, k, 3], F32, "iou")
                    dct = WT([P, k, 3], F32, "dct")
                    a2 = WT([P, k], F32, "a2")
                    cc = WT([P, k], F32, "cc")
                    nb = WT([P, k], F32, "nb")
                    mxc = WT([P, k], F32, "mxc")
                    mx = WT([P, k], F32, "mx")
                    notc = WT([P, k], F32, "notc")

                    cwx = sqt[:, :, 0:6].rearrange("p k (b f) -> p k b f", f=2)
                    cwh = sqt[:, :, 6:12].rearrange("p k (b f) -> p k b f", f=2)
                    sdcm = sqt[:, :, 12:15]
                    smdcl = sqt[:, :, 18:38]

                    if ablate == "pool":
                        # Pool workload only (+tiny DVE cc / touches)
                        nc.vector.tensor_copy(cc, confv)
                        nc.gpsimd.tensor_sub(dxy, pxy, tb[:, :, :, 0:2])
                        nc.gpsimd.tensor_sub(dclb, pcls, tcls)
                        cpx = cls_pool
                        nc.gpsimd.tensor_mul(
                            smdcl[:, :, 0:cpx], dclb[:, :, 0:cpx],
                            cc.unsqueeze(2).broadcast_to([P, k, cpx]))
                        nc.vector.tensor_copy(
                            acc[:, ci:ci + 1],
                            dxy.rearrange("p k b f -> p (k b f)")[:, 0:1])
                        nc.vector.tensor_copy(
                            acc[:, ci:ci + 1],
                            sqt.rearrange("p k d -> p (k d)")[:, 18:19])
                        continue
                    if ablate == "act":
                        # ACT workload only (+dcx on DVE as acx input)
                        t0wbx = twh0.unsqueeze(2).broadcast_to([P, k, 3, 2])
                        nc.scalar.activation(t0w2, t0wbx, AF.Copy, scale=2.0)
                        nc.scalar.activation(sp, pwh, AF.Sqrt)
                        nc.scalar.activation(st, twh, AF.Sqrt)
                        nc.vector.tensor_sub(
                            dcx, pxy,
                            txy0.unsqueeze(2).broadcast_to([P, k, 3, 2]))
                        nc.scalar.activation(acx, dcx, AF.Abs, scale=4.0 / S)
                        nc.scalar.activation(
                            WT([P, k, 35], BF16, "sqd"), o3, AF.Square,
                            accum_out=acc[:, ci:ci + 1])
                        nc.vector.tensor_copy(
                            acc[:, ci:ci + 1],
                            acx.rearrange("p k b f -> p (k b f)")[:, 0:1])
                        continue
                    if ablate == "dve":
                        # DVE workload only; Pool/ACT products substituted by
                        # cheap DVE 2x/4x ts-copies (noted as pollution)
                        nc.vector.tensor_copy(cc, confv)
                        nc.vector.tensor_sub(
                            dcx, pxy,
                            txy0.unsqueeze(2).broadcast_to([P, k, 3, 2]))
                        nc.vector.tensor_scalar_mul(t0w2, dcx, 1.0)
                        nc.vector.tensor_scalar_mul(p2w, dcx, 1.5)
                        nc.vector.tensor_scalar_mul(acx, dcx, 4.0 / S)
                        nc.vector.tensor_scalar_mul(sp, dcx, 0.5)
                        nc.vector.tensor_scalar_mul(st, t0w2, 0.5)
                        nc.vector.tensor_scalar_mul(dxy, dcx, 1.0)
                        nc.vector.tensor_sub(dclb, pcls, tcls)
                    do_pool = ablate is None
                    do_act = ablate is None

                    if do_act:
                        # -------- ACT: contiguous-ish unary mats --------
                        t0wb = twh0.unsqueeze(2).broadcast_to([P, k, 3, 2])
                        nc.scalar.activation(t0w2, t0wb, AF.Copy, scale=2.0)
                        nc.scalar.activation(p2w, pwh, AF.Copy, scale=2.0)
                        nc.scalar.activation(sp, pwh, AF.Sqrt)
                        nc.scalar.activation(st, twh, AF.Sqrt)

                    if do_pool:
                        # ------ Pool: first-touch (dxy boxes 1,2 only;
                        # box 0 == dcx box 0, reused in the cw mask) ------
                        nc.gpsimd.tensor_sub(
                            dxy[:, :, 1:3, :], pxy[:, :, 1:3, :],
                            tb[:, :, 1:3, 0:2])
                        nc.gpsimd.tensor_sub(dclb, pcls, tcls)

                    # ---------- DVE ----------
                    if ablate is None:
                        nc.vector.tensor_copy(cc, confv)
                        nc.vector.tensor_sub(
                            dcx, pxy,
                            txy0.unsqueeze(2).broadcast_to([P, k, 3, 2]))
                    if do_act:
                        # acx = |dcx| * 4/S  (contiguous ACT read, own tile)
                        nc.scalar.activation(acx, dcx, AF.Abs, scale=4.0 / S)
                    # s6d = 2pw+2tw ; mn4 = min(4pw,4tw) via 2x/4x-mode ops
                    nc.vector.tensor_add(s6d, p2w, t0w2)
                    nc.vector.tensor_scalar_mul(p4w, p2w, 2.0)
                    nc.vector.tensor_scalar_mul(t0w4, t0w2, 2.0)
                    nc.vector.tensor_tensor(mn4, p4w, t0w4, op=OP.min)
                    # u6 = s6d - acx = 4*(half-sum - |dc|) ; ov = min(mn4,u6)
                    nc.vector.tensor_sub(u6, s6d, acx)
                    nc.vector.tensor_tensor(ovf, mn4, u6, op=OP.min)
                    nc.vector.tensor_scalar_max(ovr, ovf, 0.0)
                    nc.vector.tensor_mul(inter, ovr[:, :, :, 0],
                                         ovr[:, :, :, 1])

                    # areas (x16 to match inter = 16*true), den, iou
                    nc.vector.scalar_tensor_tensor(
                        a1, ob[:, :, :, 2], 16.0, ob[:, :, :, 3],
                        op0=OP.mult, op1=OP.mult)
                    nc.vector.scalar_tensor_tensor(
                        a2, twh0[:, :, 0], 16.0, twh0[:, :, 1],
                        op0=OP.mult, op1=OP.mult)
                    nc.vector.tensor_add(
                        a12, a1, a2.unsqueeze(2).broadcast_to([P, k, 3]))
                    nc.vector.tensor_sub(den, a12, inter)
                    nc.vector.reciprocal_approx_fast(
                        rcp.rearrange("p k b -> p (k b)"),
                        den.rearrange("p k b -> p (k b)"))
                    nc.vector.tensor_mul(iou, inter, rcp)

                    # responsible-box mask: rm = (iou >= mx + (cc!=1)*BIG)
                    nc.vector.tensor_reduce(mx, iou, axis=AX.X, op=OP.max)
                    nc.vector.tensor_scalar(
                        nb, cc, 1.0, BIG, op0=OP.not_equal, op1=OP.mult)
                    nc.vector.tensor_add(mxc, mx, nb)
                    nc.vector.tensor_tensor(
                        rmb, iou, mxc.unsqueeze(2).broadcast_to([P, k, 3]),
                        op=OP.is_ge)
                    # packed sqrt(5)-scaled mask (2x-mode mat, enables 2x
                    # TT for the cw mask-muls)
                    nc.vector.tensor_scalar_mul(
                        rm6, rmb.unsqueeze(3).broadcast_to([P, k, 3, 2]), SQ5)

                    # ---------- masked residuals into SQT ----------
                    nc.vector.tensor_mul(cwx[:, :, 0:1, :], dcx[:, :, 0:1, :],
                                         rm6[:, :, 0:1, :])
                    nc.vector.tensor_mul(cwx[:, :, 1:3, :], dxy[:, :, 1:3, :],
                                         rm6[:, :, 1:3, :])
                    nc.vector.tensor_sub(dwh, sp, st)
                    nc.vector.tensor_mul(cwh, dwh, rm6)
                    nc.vector.tensor_sub(
                        dct, ob[:, :, :, 4],
                        mx.unsqueeze(2).broadcast_to([P, k, 3]))
                    nc.vector.tensor_mul(sdcm, dct, rmb)
                    nc.vector.tensor_scalar(
                        notc, cc, 1.0, SQH, op0=OP.not_equal, op1=OP.mult)
                    nc.vector.tensor_mul(
                        sqt[:, :, 15:18], ob[:, :, :, 4],
                        notc.unsqueeze(2).broadcast_to([P, k, 3]))
                    # class conf-mask, split channel-wise Pool/DVE
                    cp = cls_pool
                    ccb = cc.unsqueeze(2)
                    if cp > 0 and do_pool:
                        nc.gpsimd.tensor_mul(
                            smdcl[:, :, 0:cp], dclb[:, :, 0:cp],
                            ccb.broadcast_to([P, k, cp]))
                    if cp < 20:
                        nc.vector.tensor_mul(
                            smdcl[:, :, cp:20], dclb[:, :, cp:20],
                            ccb.broadcast_to([P, k, 20 - cp]))

                    if debug_sqt and ci == 0:
                        sq32 = WT([P, k, 38], F32, "sq32")
                        nc.vector.tensor_copy(sq32, sqt)
                        nc.sync.dma_start(
                            sqt_h.ap()[:],
                            sq32.rearrange("p k d -> p (k d)"))
                    if do_act:
                        # -------- ACT: single fused Square+accumulate ---
                        nc.scalar.activation(sqt, sqt, AF.Square,
                                             accum_out=acc[:, ci:ci + 1])
                    else:
                        nc.vector.tensor_copy(
                            acc[:, ci:ci + 1],
                            sqt.rearrange("p k d -> p (k d)")[:, 31:32])

            nc.sync.dma_start(acc_h.ap()[:], acc[:])

    nc.compile()
    return nc


_CACHE = {}


def _get_nc(bc, ks=None, io_bufs=3, loop_repeats=0, cls_pool=CLS_POOL,
            repeats=1, **_ignored):
    key = (bc, tuple(ks) if ks else None, io_bufs, loop_repeats, cls_pool,
           repeats)
    if key not in _CACHE:
        _CACHE[key] = build_nc(bc, ks, io_bufs, loop_repeats, cls_pool,
                               repeats)
    return _CACHE[key]


def combine_acc(acc_list, nchunks):
    tot = np.float64(0.0)
    for a in acc_list:
        tot += a.astype(np.float64).sum()
    return np.float32(tot / B)


BEST_KS = [49, 98, 98, 98, 49]
BEST_IO_BUFS = 3


def extra_inputs():
    return {}


def kernel(output: np.ndarray, target: np.ndarray) -> np.ndarray:
    assert output.shape == (B, S, S, D) and target.shape == (B, S, S, D)
    bc = B // NCORES
    nchunks = len(BEST_KS)
    nc = _get_nc(bc, BEST_KS, io_bufs=BEST_IO_BUFS)
    in_maps = [
        {
            "output": np.ascontiguousarray(output[i * bc:(i + 1) * bc]),
            "target": np.ascontiguousarray(target[i * bc:(i + 1) * bc]),
        }
        for i in range(NCORES)
    ]
    res = run_bass_kernel_spmd(nc, in_maps, list(range(NCORES)))
    return combine_acc([r["acc"] for r in res.results], nchunks)
